# revision 12
# baseline (speedup 1.0000x reference)
"""Self-contained Trainium2 kernel for ReRoPE sparse attention.

Problem: x(2,1024,2048) -> attention with 16 Q heads / 8 KV heads (GQA),
RoPE within a 256-token causal band, ReRoPE (query rotated at fixed
position 256, keys unrotated) outside the band, -> out proj (2048x2048).

Sharding: 8 cores = 2 batches x 4 head groups. Each core computes 4 Q
heads / 2 KV heads of one batch plus its slice of all projections, and
produces a partial (1024,2048) output (wo row-parallel). Partials are
summed on the host (the per-batch all-reduce equivalent).

Score identity used: s2 = (R_W q)@k  ==  q @ (R_{-W} k), so the fixed
ReRoPE rotation is applied once to K instead of Q (q2 is just raw q).
Head dims are de-interleaved (evens|odds) via a host-side permutation of
wq/wk columns so RoPE pairs live on partitions (p, p+64).

v2 schedule: K+Q(h0,h1) projections stream t-chunk-major against the x
DMA (PE never waits on HBM); scores are key-block-grouped with wide
query rhs into per-head exp strips; band/far select masks run on the
otherwise-idle Pool engine; Q(h2,h3)/V projections and h0..h3 scores are
zipped so the Scalar engine's exp stream keeps pace; attention+out-proj
sweep row-major with a one-row flush lag.

PSUM discipline: one pool, tag "pa" (4 slots) holds long-lived
accumulators (K/Q proj, V, attn@V), tag "pb" (4 slots) rotates
short-lived psums (scores, transposes, out-proj). 8 banks total.

All device compute in bf16 (fp32 PSUM accumulation).
"""

import numpy as np
import ml_dtypes

B, S, D = 2, 1024, 2048
NH, NKV, HD = 16, 8, 128
W = 256
HPC, KPC = 4, 2            # q heads / kv heads per core
KC = D // 128              # 16 contraction chunks
SB = S // 128              # 8 sequence blocks
SCALE = 1.0 / float(np.sqrt(HD))
BF16 = ml_dtypes.bfloat16

# band strip: key block j covers queries j..min(j+2,7)
BW = [min(3, SB - j) for j in range(SB)]            # widths (blocks)
BOFF = np.cumsum([0] + BW).tolist()                 # block offsets
# far strip: key block j covers queries j+2..7
FW = [SB - 2 - j for j in range(SB - 2)]            # widths (blocks)
FOFF = np.cumsum([0] + FW).tolist()

_NC_CACHE = {}


def _build_nc():
    import concourse.bass as bass
    import concourse.tile as tile
    from concourse import bacc, mybir
    from contextlib import ExitStack

    bf = mybir.dt.bfloat16
    f32 = mybir.dt.float32
    AF = mybir.ActivationFunctionType
    MUL = mybir.AluOpType.mult
    SUB = mybir.AluOpType.subtract

    nc = bacc.Bacc()
    xt = nc.declare_dram_parameter("xt", [D, S], bf, isOutput=False)
    wq = nc.declare_dram_parameter("wq", [D, HPC * HD], bf, isOutput=False)
    wk = nc.declare_dram_parameter("wk", [D, KPC * HD], bf, isOutput=False)
    wv = nc.declare_dram_parameter("wv", [D, KPC * HD], bf, isOutput=False)
    wo = nc.declare_dram_parameter("wo", [HPC * HD, D], bf, isOutput=False)
    tab = nc.declare_dram_parameter("tab", [128, 2 * S], bf, isOutput=False)
    cst = nc.declare_dram_parameter("cst", [128, 3 * 128], bf, isOutput=False)
    cwd = nc.declare_dram_parameter("cw", [128, 2], f32, isOutput=False)
    out = nc.declare_dram_parameter("out", [S, D], bf, isOutput=True)

    with tile.TileContext(nc) as tc:
        with ExitStack() as ctx:
            p_x = ctx.enter_context(tc.tile_pool(name="p_x", bufs=1))
            p_w = ctx.enter_context(tc.tile_pool(name="p_w", bufs=1))
            p_tab = ctx.enter_context(tc.tile_pool(name="p_tab", bufs=1))
            p_q = ctx.enter_context(tc.tile_pool(name="p_q", bufs=2 * HPC))
            p_k = ctx.enter_context(tc.tile_pool(name="p_k", bufs=2 * KPC))
            p_v = ctx.enter_context(tc.tile_pool(name="p_v", bufs=SB))
            p_ao = ctx.enter_context(tc.tile_pool(name="p_ao", bufs=HPC))
            p_e = ctx.enter_context(tc.tile_pool(name="p_e", bufs=HPC))
            p_pt = ctx.enter_context(tc.tile_pool(name="p_pt", bufs=24))
            p_an = ctx.enter_context(tc.tile_pool(name="p_an", bufs=20))
            p_kr = ctx.enter_context(tc.tile_pool(name="p_kr", bufs=2))
            p_rt = ctx.enter_context(tc.tile_pool(name="p_rt", bufs=4))
            p_rc = ctx.enter_context(tc.tile_pool(name="p_rc", bufs=4))
            p_st = ctx.enter_context(tc.tile_pool(name="p_st", bufs=2))

            ps = ctx.enter_context(
                tc.tile_pool(name="ps", bufs=4, space="PSUM"))

            # ---- DMA schedule ----
            # ring A (sync): x t0..15 singles, wq23, wo, then row outputs
            # ring B (scalar): wk/wq01 t-pair interleave, wv, tab/cst/cw
            xt_sb = p_x.tile([128, KC * S], bf, tag="xt")
            xt_d = xt.ap().rearrange("(t p) s -> p t s", p=128)
            xt_v = xt_sb[:].rearrange("p (t s) -> p t s", t=KC)
            wq_sb = p_w.tile([128, KC * HPC * HD], bf, tag="wq")
            wq_d = wq.ap().rearrange("(t p) c -> p t c", p=128)
            wq_v = wq_sb[:].rearrange("p (t c) -> p t c", t=KC)
            wk_sb = p_w.tile([128, KC * KPC * HD], bf, tag="wk")
            wk_d = wk.ap().rearrange("(t p) c -> p t c", p=128)
            wk_v = wk_sb[:].rearrange("p (t c) -> p t c", t=KC)
            wv_sb = p_w.tile([128, KC * KPC * HD], bf, tag="wv")
            wv_d = wv.ap().rearrange("(t p) c -> p t c", p=128)
            wv_v = wv_sb[:].rearrange("p (t c) -> p t c", t=KC)
            wo_sb = p_w.tile([128, HPC * D], bf, tag="wo")
            wo_d = wo.ap().rearrange("(t p) c -> p t c", p=128)
            wo_v = wo_sb[:].rearrange("p (t c) -> p t c", t=HPC)

            # ring B first: wk + wq(h0,h1) interleaved by t-pairs
            for tp in range(KC // 2):
                nc.scalar.dma_start(wk_v[:, 2 * tp:2 * tp + 2, :],
                                    wk_d[:, 2 * tp:2 * tp + 2, :])
                nc.scalar.dma_start(wq_v[:, 2 * tp:2 * tp + 2, 0:256],
                                    wq_d[:, 2 * tp:2 * tp + 2, 0:256])
            # ring A: x chunks in t order
            for t in range(KC):
                nc.sync.dma_start(xt_v[:, t:t + 1, :], xt_d[:, t:t + 1, :])
            # ring B continues: wv, then tables
            nc.scalar.dma_start(wv_v[:, 0:8, :], wv_d[:, 0:8, :])
            nc.scalar.dma_start(wv_v[:, 8:16, :], wv_d[:, 8:16, :])
            tab_sb = p_tab.tile([128, 2 * S], bf, tag="tab")
            nc.scalar.dma_start(tab_sb[:], tab[:, :])
            cst_sb = p_tab.tile([128, 3 * 128], bf, tag="cst")
            nc.scalar.dma_start(cst_sb[:], cst[:, :])
            cw_sb = p_tab.tile([128, 2], f32, tag="cw")
            nc.scalar.dma_start(cw_sb[:], cwd[:, :])
            # ring A continues: wq(h2,h3), wo
            for tp in range(KC // 2):
                nc.sync.dma_start(wq_v[:, 2 * tp:2 * tp + 2, 256:512],
                                  wq_d[:, 2 * tp:2 * tp + 2, 256:512])
            nc.sync.dma_start(wo_v[:, 0:2, :], wo_d[:, 0:2, :])
            nc.sync.dma_start(wo_v[:, 2:4, :], wo_d[:, 2:4, :])

            def xts(t, lo, hi):
                return xt_sb[:, t * S + lo:t * S + hi]

            cosT = tab_sb[:, 0:S]
            sinT = tab_sb[:, S:2 * S]     # top half negated (host-side)
            m0_t = cst_sb[:, 0:128]      # (k <= q)
            m2_t = cst_sb[:, 128:256]    # (q < k)
            id_t = cst_sb[:, 256:384]

            def rope_var(dst, src):
                """Positional rope over full S; pairs on (p, p+64).
                Table halves carry signs: sinT[0:64]=-sin, [64:128]=+sin."""
                r, i = src[0:64, :], src[64:128, :]
                t1 = p_rt.tile([64, S], bf, tag="rt")
                nc.vector.tensor_mul(t1[:], r, cosT[0:64, :])
                t2 = p_rt.tile([64, S], bf, tag="rt")
                nc.vector.tensor_mul(t2[:], i, sinT[64:128, :])
                nc.vector.tensor_sub(dst[0:64, :], t1[:], t2[:])
                t3 = p_rt.tile([64, S], bf, tag="rt")
                nc.vector.tensor_mul(t3[:], r, sinT[0:64, :])    # = -r*sin
                t4 = p_rt.tile([64, S], bf, tag="rt")
                nc.vector.tensor_mul(t4[:], i, cosT[64:128, :])
                nc.vector.tensor_sub(dst[64:128, :], t4[:], t3[:])

            def rope_negw(dst, src):
                """R_{-W}: or = r*cw + i*sw, oi = i*cw - r*sw.
                cw col0 = [cw;cw], col1 = [+sw; -sw]. stt inputs must
                share a start partition (verifier), outputs may differ."""
                r, i = src[0:64, :], src[64:128, :]
                t2 = p_rt.tile([64, S], bf, tag="rt")
                nc.vector.tensor_scalar_mul(t2[:], i, cw_sb[64:128, 1:2])
                nc.vector.scalar_tensor_tensor(
                    dst[0:64, :], r, cw_sb[0:64, 0:1], t2[:], MUL, SUB)
                t4 = p_rt.tile([128, S], bf, tag="rt4", bufs=2)
                nc.vector.tensor_scalar_mul(t4[64:128, :], r,
                                            cw_sb[0:64, 1:2])
                nc.vector.scalar_tensor_tensor(
                    dst[64:128, :], i, cw_sb[64:128, 0:1], t4[64:128, :],
                    MUL, SUB)

            # ---- phase A: K + Q(h0,h1) projections, t-chunk-major ----
            kps = [[ps.tile([128, 512], f32, tag="pa",
                            name=f"kps{kv}{half}") for half in range(2)]
                   for kv in range(KPC)]
            qps01 = [[ps.tile([128, 512], f32, tag=("sc" if h == 0 else "pv"),
                              bufs=2, name=f"qps{h}{half}")
                      for half in range(2)]
                     for h in range(2)]
            for t in range(KC):
                for kv in range(KPC):
                    for half in range(2):
                        nc.tensor.matmul(
                            kps[kv][half][:],
                            lhsT=wk_sb[:, t * 256 + kv * 128:
                                       t * 256 + (kv + 1) * 128],
                            rhs=xts(t, half * 512, (half + 1) * 512),
                            start=(t == 0), stop=(t == KC - 1))
                for h in range(2):
                    for half in range(2):
                        nc.tensor.matmul(
                            qps01[h][half][:],
                            lhsT=wq_sb[:, t * 512 + h * 128:
                                       t * 512 + (h + 1) * 128],
                            rhs=xts(t, half * 512, (half + 1) * 512),
                            start=(t == 0), stop=(t == KC - 1))

            # epilogue: psum -> sbuf copies (scalar), ropes (vector)
            k1_t, k2_t = [], []
            for kv in range(KPC):
                kr = p_kr.tile([128, S], bf, tag="kr")
                for half in range(2):
                    nc.scalar.copy(kr[:, half * 512:(half + 1) * 512],
                                   kps[kv][half][:])
                d1 = p_k.tile([128, S], bf, tag="k")
                d2 = p_k.tile([128, S], bf, tag="k")
                rope_var(d1, kr)
                rope_negw(d2, kr)
                k1_t.append(d1)
                k2_t.append(d2)

            q1_t, q2_t = [], []
            for h in range(2):
                d2 = p_q.tile([128, S], bf, tag="q")
                for half in range(2):
                    nc.scalar.copy(d2[:, half * 512:(half + 1) * 512],
                                   qps01[h][half][:])
                d1 = p_q.tile([128, S], bf, tag="q")
                rope_var(d1, d2)
                q1_t.append(d1)
                q2_t.append(d2)

            # ---- score machinery (key-block-grouped, wide query rhs) ----
            eb_t = {}  # h -> band strip
            ef_t = {}  # h -> far strip

            def score_mms(h):
                """List of thunks, each emitting one band/far matmul+exp."""
                kv = h // 2
                ebs = p_e.tile([128, BOFF[-1] * 128], bf, tag="eb",
                               name=f"eb{h}")
                efs = p_e.tile([128, FOFF[-1] * 128], bf, tag="ef",
                               name=f"ef{h}")
                eb_t[h] = ebs
                ef_t[h] = efs

                def band(j):
                    n = BW[j] * 128
                    pb = ps.tile([128, n], f32, tag="sc", bufs=2, name="scb")
                    nc.tensor.matmul(
                        pb[:], lhsT=k1_t[kv][:, j * 128:(j + 1) * 128],
                        rhs=q1_t[h][:, j * 128:j * 128 + n],
                        start=True, stop=True)
                    o = BOFF[j] * 128
                    nc.scalar.activation(ebs[:, o:o + n], pb[:],
                                         AF.Exp, scale=SCALE)

                def far(j, qlo, qn):
                    pb = ps.tile([128, qn * 128], f32, tag="sc", bufs=2,
                                 name="scf")
                    nc.tensor.matmul(
                        pb[:], lhsT=k2_t[kv][:, j * 128:(j + 1) * 128],
                        rhs=q2_t[h][:, qlo * 128:(qlo + qn) * 128],
                        start=True, stop=True)
                    o = (FOFF[j] + qlo - (j + 2)) * 128
                    nc.scalar.activation(efs[:, o:o + qn * 128], pb[:],
                                         AF.Exp, scale=SCALE)

                thunks = []
                for j in range(SB):
                    thunks.append(lambda j=j: band(j))
                    if j < SB - 2:
                        qlo = j + 2
                        while qlo < SB:
                            qn = min(4, SB - qlo)
                            thunks.append(
                                lambda j=j, qlo=qlo, qn=qn: far(j, qlo, qn))
                            qlo += qn
                return thunks

            def band_blk(h, j, i):
                o = (BOFF[j] + i - j) * 128
                return eb_t[h][:, o:o + 128]

            def far_blk(h, j, i):
                o = (FOFF[j] + i - (j + 2)) * 128
                return ef_t[h][:, o:o + 128]

            # masks (gpsimd): p0 = diag*m0 ; pd = band*m2 + far*m0
            pmask = {}

            def emit_masks(i, h):
                p0 = p_pt.tile([128, 128], bf, tag="pt")
                nc.gpsimd.tensor_mul(p0[:], band_blk(h, i, i), m0_t)
                pd = None
                if i >= 2:
                    pa_ = p_pt.tile([128, 128], bf, tag="pt")
                    nc.gpsimd.tensor_mul(pa_[:], band_blk(h, i - 2, i), m2_t)
                    pd = p_pt.tile([128, 128], bf, tag="pt")
                    nc.gpsimd.tensor_mul(pd[:], far_blk(h, i - 2, i), m0_t)
                    nc.gpsimd.tensor_add(pd[:], pd[:], pa_[:])
                pmask[(i, h)] = (p0, pd)

            def P(i, h, j):
                p0, pd = pmask[(i, h)]
                if j == i:
                    return p0[:]
                if j == i - 2 and i >= 2:
                    return pd[:]
                if j >= i - 2:
                    return band_blk(h, j, i)
                return far_blk(h, j, i)

            # ---- phase B1: Q(h2,h3) proj, zip scores h0 into tail ----
            qps23 = [[ps.tile([128, 512], f32, tag="pa",
                              name=f"qps{h}{half}") for half in range(2)]
                     for h in range(2, 4)]

            def q23_mm(t, h, half):
                nc.tensor.matmul(
                    qps23[h - 2][half][:],
                    lhsT=wq_sb[:, t * 512 + h * 128:t * 512 + (h + 1) * 128],
                    rhs=xts(t, half * 512, (half + 1) * 512),
                    start=(t == 0), stop=(t == KC - 1))

            q23_seq = [(t, h, half) for t in range(KC)
                       for h in (2, 3) for half in range(2)]
            for (t, h, half) in q23_seq[:32]:
                q23_mm(t, h, half)
            sc0 = score_mms(0)
            sc1 = score_mms(1)
            sco = sc0 + sc1
            sci = 0
            for n, (t, h, half) in enumerate(q23_seq[32:]):
                q23_mm(t, h, half)
                if n % 2 == 1 and sci < len(sc0):
                    sco[sci]()
                    sci += 1

            # q23 epilogue (vector ropes queued ahead of v copies)
            for h in (2, 3):
                d2 = p_q.tile([128, S], bf, tag="q")
                for half in range(2):
                    nc.scalar.copy(d2[:, half * 512:(half + 1) * 512],
                                   qps23[h - 2][half][:])
                d1 = p_q.tile([128, S], bf, tag="q")
                rope_var(d1, d2)
                q1_t.append(d1)
                q2_t.append(d2)

            # ---- phase B2: V proj, zip scores h1 ----
            v_t = []
            for sb in range(SB):
                pv = ps.tile([128, KPC * HD], f32, tag="pv", bufs=2,
                             name="vps")
                for t in range(KC):
                    nc.tensor.matmul(
                        pv[:],
                        lhsT=xts(t, sb * 128, (sb + 1) * 128),
                        rhs=wv_sb[:, t * 256:(t + 1) * 256],
                        start=(t == 0), stop=(t == KC - 1))
                tv = p_v.tile([128, 2 * (HD + 1)], bf, tag="v")
                nc.vector.tensor_copy(tv[:, 0:HD], pv[:, 0:HD])
                nc.vector.tensor_copy(tv[:, HD + 1:2 * HD + 1],
                                      pv[:, HD:2 * HD])
                nc.vector.memset(tv[:, HD:HD + 1], 1.0)
                nc.vector.memset(tv[:, 2 * HD + 1:2 * HD + 2], 1.0)
                v_t.append(tv)
                for _ in range(2):
                    if sci < len(sco):
                        sco[sci]()
                        sci += 1
            while sci < len(sco):
                sco[sci]()
                sci += 1

            # masks for h0/h1 (consumed by the attnv zip below)
            for i in range(SB):
                for h in (0, 1):
                    emit_masks(i, h)

            an_t = {}

            def emit_attnv_mm(i, h):
                kv = h // 2
                pso = ps.tile([128, HD + 1], f32, tag="pa", name="pso")
                for j in range(i + 1):
                    nc.tensor.matmul(
                        pso[:], lhsT=P(i, h, j),
                        rhs=v_t[j][:, kv * (HD + 1):(kv + 1) * (HD + 1)],
                        start=(j == 0), stop=(j == i))
                rc = p_rc.tile([128, 1], f32, tag="rc")
                nc.vector.reciprocal(rc[:], pso[:, HD:HD + 1])
                an = p_an.tile([128, 128], bf, tag="an")
                nc.vector.tensor_scalar_mul(an[:], pso[:, 0:HD], rc[:])
                an_t[(i, h)] = an

            # ---- zip: scores h2+h3 with attnv rows of h0/h1 ----
            sc23 = score_mms(2) + score_mms(3)
            av01 = [(i, h) for i in range(SB) for h in (0, 1)]
            avi = 0
            for n, th in enumerate(sc23):
                th()
                if n % 2 == 1 and avi < len(av01):
                    emit_attnv_mm(*av01[avi])
                    avi += 1
            while avi < len(av01):
                emit_attnv_mm(*av01[avi])
                avi += 1

            for i in range(SB):
                for h in (2, 3):
                    emit_masks(i, h)

            # ---- row sweep: attnv h2/h3 + transposes + out-proj ----
            def emit_transpose(i, h):
                pst = ps.tile([128, 128], bf, tag="pv", bufs=2, name="pst")
                nc.tensor.transpose(pst[:], an_t[(i, h)][:], id_t)
                nc.vector.tensor_copy(ao_t[h][:, i * 128:(i + 1) * 128],
                                      pst[:])

            def emit_outproj(i):
                st = p_st.tile([128, D], bf, tag="st")
                for cg in range(4):
                    po = ps.tile([128, 512], f32, tag="sc", bufs=2, name="po")
                    for hc in range(HPC):
                        nc.tensor.matmul(
                            po[:],
                            lhsT=ao_t[hc][:, i * 128:(i + 1) * 128],
                            rhs=wo_sb[:, hc * D + cg * 512:
                                      hc * D + (cg + 1) * 512],
                            start=(hc == 0), stop=(hc == HPC - 1))
                    if cg % 2 == 0:
                        nc.vector.tensor_copy(
                            st[:, cg * 512:(cg + 1) * 512], po[:])
                    else:
                        nc.scalar.copy(
                            st[:, cg * 512:(cg + 1) * 512], po[:])
                nc.sync.dma_start(out[i * 128:(i + 1) * 128, :], st[:])

            ao_t = [p_ao.tile([128, S], bf, tag="ao", name=f"ao{h}")
                    for h in range(HPC)]
            for i in range(SB):
                emit_attnv_mm(i, 2)
                emit_attnv_mm(i, 3)
                if i > 0:
                    for h in range(HPC):
                        emit_transpose(i - 1, h)
                    emit_outproj(i - 1)
            for h in range(HPC):
                emit_transpose(SB - 1, h)
            emit_outproj(SB - 1)

    nc.finalize()
    return nc


def _get_nc():
    if "nc" not in _NC_CACHE:
        _NC_CACHE["nc"] = _build_nc()
    return _NC_CACHE["nc"]


def _host_inputs(x, freqs_cos, freqs_sin, wq, wk, wv, wo):
    """Build the 8 per-core input maps (host-side shard + layout prep)."""
    x = np.asarray(x, np.float32)
    wq = np.asarray(wq, np.float32)
    wk = np.asarray(wk, np.float32)
    wv = np.asarray(wv, np.float32)
    wo = np.asarray(wo, np.float32)
    perm = np.concatenate([np.arange(0, HD, 2), np.arange(1, HD, 2)])

    cos_t = np.asarray(freqs_cos, np.float32).T        # (64, S)
    sin_t = np.asarray(freqs_sin, np.float32).T
    top = np.concatenate([cos_t, -sin_t], axis=1)      # (64, 2S)
    bot = np.concatenate([cos_t, sin_t], axis=1)
    tab = np.ascontiguousarray(np.concatenate([top, bot], axis=0)).astype(BF16)
    ki = np.arange(128)[:, None]
    qi = np.arange(128)[None, :]
    m0 = (ki <= qi).astype(BF16)                       # causal / far-select
    m2 = (qi < ki).astype(BF16)                        # in-band select (d=2)
    ident = np.eye(128, dtype=BF16)
    cstm = np.ascontiguousarray(np.concatenate([m0, m2, ident], axis=1))

    wq3 = wq.reshape(D, NH, HD)
    wk3 = wk.reshape(D, NKV, HD)
    wv3 = wv.reshape(D, NKV, HD)
    wo3 = wo.reshape(NH, HD, D)

    in_maps = []
    for c in range(8):
        b, g = divmod(c, 4)
        wqc = wq3[:, 4 * g:4 * g + 4][:, :, perm].reshape(D, HPC * HD)
        wkc = wk3[:, 2 * g:2 * g + 2][:, :, perm].reshape(D, KPC * HD)
        wvc = wv3[:, 2 * g:2 * g + 2].reshape(D, KPC * HD)
        woc = wo3[4 * g:4 * g + 4].reshape(HPC * HD, D)
        in_maps.append({
            "xt": np.ascontiguousarray(x[b].T).astype(BF16),
            "wq": np.ascontiguousarray(wqc).astype(BF16),
            "wk": np.ascontiguousarray(wkc).astype(BF16),
            "wv": np.ascontiguousarray(wvc).astype(BF16),
            "wo": np.ascontiguousarray(woc).astype(BF16),
            "tab": tab, "cst": cstm,
            "cw": np.ascontiguousarray(np.concatenate([
                np.stack([cos_t[:, W], sin_t[:, W]], axis=1),
                np.stack([cos_t[:, W], -sin_t[:, W]], axis=1),
            ], axis=0)).astype(np.float32),
        })
    return in_maps


def _run(nc, in_maps, **kw):
    from concourse.bass_utils import run_bass_kernel_spmd
    return run_bass_kernel_spmd(nc, in_maps, core_ids=list(range(8)), **kw)


def kernel(x, freqs_cos, freqs_sin, wq, wk, wv, wo):
    nc = _get_nc()
    in_maps = _host_inputs(x, freqs_cos, freqs_sin, wq, wk, wv, wo)
    res = _run(nc, in_maps)
    parts = [np.asarray(res.results[c]["out"], np.float32) for c in range(8)]
    out = np.stack([sum(parts[0:4]), sum(parts[4:8])])
    return out.astype(np.float32)


# revision 16
# speedup vs baseline: 1.0396x; 1.0396x over previous
"""Self-contained Trainium2 kernel for ReRoPE sparse attention.

Problem: x(2,1024,2048) -> attention with 16 Q heads / 8 KV heads (GQA),
RoPE within a 256-token causal band, ReRoPE (query rotated at fixed
position 256, keys unrotated) outside the band, -> out proj (2048x2048).

Sharding: 8 cores = 2 batches x 4 head groups. Each core computes 4 Q
heads / 2 KV heads of one batch plus its slice of all projections, and
produces a partial (1024,2048) output (wo row-parallel). Partials are
summed on the host (the per-batch all-reduce equivalent).

Score identity used: s2 = (R_W q)@k  ==  q @ (R_{-W} k), so the fixed
ReRoPE rotation is applied once to K instead of Q (q2 is just raw q).
Head dims are de-interleaved (evens|odds) via a host-side permutation of
wq/wk columns so RoPE pairs live on partitions (p, p+64).

v4 schedule: projections complete staggered (K, then Q heads h0..h3,
then V) so psum->sbuf copies and ropes overlap later projections;
key-block-grouped scores with wide query rhs stream zipped into the
Q(h3)/V matmul stream so the Scalar engine's exp pace is matched;
band/far select masks split across Vector (h0,h1) and Pool (h2,h3);
attention h0/h1 zipped behind the last scores; row sweep does attention
h2/h3 + lagged transposes + out-proj + one merged DMA per row.

PSUM tags: pa(4) K psums -> Q h2/h3 -> attn@V accumulators; sc(2)
Q h0 -> score psums -> out-proj; pv(2) Q h1 -> V -> transposes.

All device compute in bf16 (fp32 PSUM accumulation).
"""

import numpy as np
import ml_dtypes

B, S, D = 2, 1024, 2048
NH, NKV, HD = 16, 8, 128
W = 256
HPC, KPC = 4, 2            # q heads / kv heads per core
KC = D // 128              # 16 contraction chunks
SB = S // 128              # 8 sequence blocks
SCALE = 1.0 / float(np.sqrt(HD))
BF16 = ml_dtypes.bfloat16

# band strip: key block j covers queries j..min(j+2,7)
BW = [min(3, SB - j) for j in range(SB)]            # widths (blocks)
BOFF = np.cumsum([0] + BW).tolist()                 # block offsets
# far strip: key block j covers queries j+2..7
FW = [SB - 2 - j for j in range(SB - 2)]            # widths (blocks)
FOFF = np.cumsum([0] + FW).tolist()

_NC_CACHE = {}


def _build_nc():
    import concourse.bass as bass
    import concourse.tile as tile
    from concourse import bacc, mybir
    from contextlib import ExitStack

    bf = mybir.dt.bfloat16
    f32 = mybir.dt.float32
    AF = mybir.ActivationFunctionType
    MUL = mybir.AluOpType.mult
    SUB = mybir.AluOpType.subtract

    nc = bacc.Bacc()
    xt = nc.declare_dram_parameter("xt", [D, S], bf, isOutput=False)
    wq = nc.declare_dram_parameter("wq", [D, HPC * HD], bf, isOutput=False)
    wk = nc.declare_dram_parameter("wk", [D, KPC * HD], bf, isOutput=False)
    wv = nc.declare_dram_parameter("wv", [D, KPC * HD], bf, isOutput=False)
    wo = nc.declare_dram_parameter("wo", [HPC * HD, D], bf, isOutput=False)
    tab = nc.declare_dram_parameter("tab", [128, 2 * S], bf, isOutput=False)
    cst = nc.declare_dram_parameter("cst", [128, 3 * 128], bf, isOutput=False)
    cwd = nc.declare_dram_parameter("cw", [128, 2], f32, isOutput=False)
    out = nc.declare_dram_parameter("out", [S, D], bf, isOutput=True)

    with tile.TileContext(nc) as tc:
        with ExitStack() as ctx:
            p_x = ctx.enter_context(tc.tile_pool(name="p_x", bufs=1))
            p_w = ctx.enter_context(tc.tile_pool(name="p_w", bufs=1))
            p_tab = ctx.enter_context(tc.tile_pool(name="p_tab", bufs=1))
            p_q = ctx.enter_context(tc.tile_pool(name="p_q", bufs=2 * HPC))
            p_k = ctx.enter_context(tc.tile_pool(name="p_k", bufs=2 * KPC))
            p_v = ctx.enter_context(tc.tile_pool(name="p_v", bufs=SB))
            p_ao = ctx.enter_context(tc.tile_pool(name="p_ao", bufs=HPC))
            p_e = ctx.enter_context(tc.tile_pool(name="p_e", bufs=HPC))
            p_pt = ctx.enter_context(tc.tile_pool(name="p_pt", bufs=24))
            p_an = ctx.enter_context(tc.tile_pool(name="p_an", bufs=20))
            p_kr = ctx.enter_context(tc.tile_pool(name="p_kr", bufs=2))
            p_rt = ctx.enter_context(tc.tile_pool(name="p_rt", bufs=4))
            p_rc = ctx.enter_context(tc.tile_pool(name="p_rc", bufs=4))
            p_st = ctx.enter_context(tc.tile_pool(name="p_st", bufs=2))

            ps = ctx.enter_context(
                tc.tile_pool(name="ps", bufs=4, space="PSUM"))

            # ---- DMA schedule (x striped across both rings) ----
            xt_sb = p_x.tile([128, KC * S], bf, tag="xt")
            xt_d = xt.ap().rearrange("(t p) s -> p t s", p=128)
            xt_v = xt_sb[:].rearrange("p (t s) -> p t s", t=KC)
            wq_sb = p_w.tile([128, KC * HPC * HD], bf, tag="wq")
            wq_d = wq.ap().rearrange("(t p) c -> p t c", p=128)
            wq_v = wq_sb[:].rearrange("p (t c) -> p t c", t=KC)
            wk_sb = p_w.tile([128, KC * KPC * HD], bf, tag="wk")
            wk_d = wk.ap().rearrange("(t p) c -> p t c", p=128)
            wk_v = wk_sb[:].rearrange("p (t c) -> p t c", t=KC)
            wv_sb = p_w.tile([128, KC * KPC * HD], bf, tag="wv")
            wv_d = wv.ap().rearrange("(t p) c -> p t c", p=128)
            wv_v = wv_sb[:].rearrange("p (t c) -> p t c", t=KC)
            wo_sb = p_w.tile([128, HPC * D], bf, tag="wo")
            wo_d = wo.ap().rearrange("(t p) c -> p t c", p=128)
            wo_v = wo_sb[:].rearrange("p (t c) -> p t c", t=HPC)

            # ring A (sync): wk pairs + even x chunks, then wo
            nc.sync.dma_start(wk_v[:, 0:2, :], wk_d[:, 0:2, :])
            nc.sync.dma_start(wk_v[:, 2:4, :], wk_d[:, 2:4, :])
            nc.sync.dma_start(xt_v[:, 0:1, :], xt_d[:, 0:1, :])
            nc.sync.dma_start(xt_v[:, 2:3, :], xt_d[:, 2:3, :])
            nc.sync.dma_start(wk_v[:, 4:6, :], wk_d[:, 4:6, :])
            nc.sync.dma_start(wk_v[:, 6:8, :], wk_d[:, 6:8, :])
            nc.sync.dma_start(xt_v[:, 4:5, :], xt_d[:, 4:5, :])
            nc.sync.dma_start(xt_v[:, 6:7, :], xt_d[:, 6:7, :])
            nc.sync.dma_start(wk_v[:, 8:12, :], wk_d[:, 8:12, :])
            nc.sync.dma_start(xt_v[:, 8:9, :], xt_d[:, 8:9, :])
            nc.sync.dma_start(xt_v[:, 10:11, :], xt_d[:, 10:11, :])
            nc.sync.dma_start(wk_v[:, 12:16, :], wk_d[:, 12:16, :])
            nc.sync.dma_start(xt_v[:, 12:13, :], xt_d[:, 12:13, :])
            nc.sync.dma_start(xt_v[:, 14:15, :], xt_d[:, 14:15, :])
            nc.sync.dma_start(wo_v[:, 0:2, :], wo_d[:, 0:2, :])
            nc.sync.dma_start(wo_v[:, 2:4, :], wo_d[:, 2:4, :])
            # ring B (scalar): odd x chunks, tables, per-head wq, wv
            for t in range(1, KC, 2):
                nc.scalar.dma_start(xt_v[:, t:t + 1, :], xt_d[:, t:t + 1, :])
            tab_sb = p_tab.tile([128, 2 * S], bf, tag="tab")
            nc.scalar.dma_start(tab_sb[:], tab[:, :])
            cw_sb = p_tab.tile([128, 2], f32, tag="cw")
            nc.scalar.dma_start(cw_sb[:], cwd[:, :])
            for h in range(2):
                nc.scalar.dma_start(wq_v[:, :, h * 128:(h + 1) * 128],
                                    wq_d[:, :, h * 128:(h + 1) * 128])
            cst_sb = p_tab.tile([128, 3 * 128], bf, tag="cst")
            nc.scalar.dma_start(cst_sb[:], cst[:, :])
            for h in range(2, 4):
                nc.scalar.dma_start(wq_v[:, :, h * 128:(h + 1) * 128],
                                    wq_d[:, :, h * 128:(h + 1) * 128])
            nc.scalar.dma_start(wv_v[:, 0:8, :], wv_d[:, 0:8, :])
            nc.scalar.dma_start(wv_v[:, 8:16, :], wv_d[:, 8:16, :])

            def xts(t, lo, hi):
                return xt_sb[:, t * S + lo:t * S + hi]

            cosT = tab_sb[:, 0:S]
            sinT = tab_sb[:, S:2 * S]     # top half negated (host-side)
            m0_t = cst_sb[:, 0:128]      # (k <= q)
            m2_t = cst_sb[:, 128:256]    # (q < k)
            id_t = cst_sb[:, 256:384]

            def rope_var(dst, src):
                """Positional rope over full S; pairs on (p, p+64).
                Table halves carry signs: sinT[0:64]=-sin, [64:128]=+sin."""
                r, i = src[0:64, :], src[64:128, :]
                t1 = p_rt.tile([64, S], bf, tag="rt")
                nc.vector.tensor_mul(t1[:], r, cosT[0:64, :])
                t2 = p_rt.tile([64, S], bf, tag="rt")
                nc.vector.tensor_mul(t2[:], i, sinT[64:128, :])
                nc.vector.tensor_sub(dst[0:64, :], t1[:], t2[:])
                t3 = p_rt.tile([64, S], bf, tag="rt")
                nc.vector.tensor_mul(t3[:], r, sinT[0:64, :])    # = -r*sin
                t4 = p_rt.tile([64, S], bf, tag="rt")
                nc.vector.tensor_mul(t4[:], i, cosT[64:128, :])
                nc.vector.tensor_sub(dst[64:128, :], t4[:], t3[:])

            def rope_negw(dst, src):
                """R_{-W}: or = r*cw + i*sw, oi = i*cw - r*sw.
                cw col0 = [cw;cw], col1 = [+sw; -sw]. stt inputs must
                share a start partition (verifier), outputs may differ."""
                r, i = src[0:64, :], src[64:128, :]
                t2 = p_rt.tile([64, S], bf, tag="rt")
                nc.vector.tensor_scalar_mul(t2[:], i, cw_sb[64:128, 1:2])
                nc.vector.scalar_tensor_tensor(
                    dst[0:64, :], r, cw_sb[0:64, 0:1], t2[:], MUL, SUB)
                t4 = p_rt.tile([128, S], bf, tag="rt4", bufs=2)
                nc.vector.tensor_scalar_mul(t4[64:128, :], r,
                                            cw_sb[0:64, 1:2])
                nc.vector.scalar_tensor_tensor(
                    dst[64:128, :], i, cw_sb[64:128, 0:1], t4[64:128, :],
                    MUL, SUB)

            # ---- K projection (finishes early; ropes overlap Q) ----
            kps = [[ps.tile([128, 512], f32, tag="pa",
                            name=f"kps{kv}{half}") for half in range(2)]
                   for kv in range(KPC)]
            for t in range(KC):
                for kv in range(KPC):
                    for half in range(2):
                        nc.tensor.matmul(
                            kps[kv][half][:],
                            lhsT=wk_sb[:, t * 256 + kv * 128:
                                       t * 256 + (kv + 1) * 128],
                            rhs=xts(t, half * 512, (half + 1) * 512),
                            start=(t == 0), stop=(t == KC - 1))

            k1_t, k2_t = [], []
            for kv in range(KPC):
                kr = p_kr.tile([128, S], bf, tag="kr")
                for half in range(2):
                    nc.scalar.copy(kr[:, half * 512:(half + 1) * 512],
                                   kps[kv][half][:])
                d1 = p_k.tile([128, S], bf, tag="k")
                d2 = p_k.tile([128, S], bf, tag="k")
                rope_var(d1, kr)
                rope_negw(d2, kr)
                k1_t.append(d1)
                k2_t.append(d2)

            # ---- score machinery (key-block-grouped, wide query rhs) ----
            eb_t = {}
            ef_t = {}
            q1_t, q2_t = [], []

            for _h in range(HPC):
                eb_t[_h] = p_e.tile([128, BOFF[-1] * 128], bf, tag="eb",
                                    name=f"eb{_h}")
                ef_t[_h] = p_e.tile([128, FOFF[-1] * 128], bf, tag="ef",
                                    name=f"ef{_h}")

            def score_mms(h):
                """List of thunks, each emitting one band/far matmul+exp."""
                kv = h // 2
                ebs = eb_t[h]
                efs = ef_t[h]

                def band(j):
                    n = BW[j] * 128
                    pb = ps.tile([128, n], f32, tag="sc", bufs=2, name="scb")
                    nc.tensor.matmul(
                        pb[:], lhsT=k1_t[kv][:, j * 128:(j + 1) * 128],
                        rhs=q1_t[h][:, j * 128:j * 128 + n],
                        start=True, stop=True)
                    o = BOFF[j] * 128
                    nc.scalar.activation(ebs[:, o:o + n], pb[:],
                                         AF.Exp, scale=SCALE)

                def far(j, qlo, qn):
                    pb = ps.tile([128, qn * 128], f32, tag="sc", bufs=2,
                                 name="scf")
                    nc.tensor.matmul(
                        pb[:], lhsT=k2_t[kv][:, j * 128:(j + 1) * 128],
                        rhs=q2_t[h][:, qlo * 128:(qlo + qn) * 128],
                        start=True, stop=True)
                    o = (FOFF[j] + qlo - (j + 2)) * 128
                    nc.scalar.activation(efs[:, o:o + qn * 128], pb[:],
                                         AF.Exp, scale=SCALE)

                thunks = []
                for j in range(SB):
                    thunks.append(lambda j=j: band(j))
                    if j < SB - 2:
                        qlo = j + 2
                        while qlo < SB:
                            qn = min(4, SB - qlo)
                            thunks.append(
                                lambda j=j, qlo=qlo, qn=qn: far(j, qlo, qn))
                            qlo += qn
                return thunks

            def band_blk(h, j, i):
                o = (BOFF[j] + i - j) * 128
                return eb_t[h][:, o:o + 128]

            def far_blk(h, j, i):
                o = (FOFF[j] + i - (j + 2)) * 128
                return ef_t[h][:, o:o + 128]

            # masks: p0 = diag*m0 ; pd = band*m2 + far*m0
            pmask = {}

            def emit_masks(i, h, eng):
                p0 = p_pt.tile([128, 128], bf, tag="pt")
                eng.tensor_mul(p0[:], band_blk(h, i, i), m0_t)
                pd = None
                if i >= 2:
                    pa_ = p_pt.tile([128, 128], bf, tag="pt")
                    eng.tensor_mul(pa_[:], band_blk(h, i - 2, i), m2_t)
                    pd = p_pt.tile([128, 128], bf, tag="pt")
                    eng.tensor_mul(pd[:], far_blk(h, i - 2, i), m0_t)
                    eng.tensor_add(pd[:], pd[:], pa_[:])
                pmask[(i, h)] = (p0, pd)

            def P(i, h, j):
                p0, pd = pmask[(i, h)]
                if j == i:
                    return p0[:]
                if j == i - 2 and i >= 2:
                    return pd[:]
                if j >= i - 2:
                    return band_blk(h, j, i)
                return far_blk(h, j, i)

            # ---- Q projections, one head at a time ----
            QTAG = {0: ("sc", 2), 1: ("pv", 2), 2: ("pa", 4), 3: ("pa", 4)}

            def q_proj(h, zipped=()):
                tag, nb = QTAG[h]
                qps = [ps.tile([128, 512], f32, tag=tag, bufs=nb,
                               name=f"qps{h}{half}") for half in range(2)]
                zi = iter(zipped)
                for t in range(KC):
                    for half in range(2):
                        nc.tensor.matmul(
                            qps[half][:],
                            lhsT=wq_sb[:, t * 512 + h * 128:
                                       t * 512 + (h + 1) * 128],
                            rhs=xts(t, half * 512, (half + 1) * 512),
                            start=(t == 0), stop=(t == KC - 1))
                    if t % 2 == 1:
                        th = next(zi, None)
                        if th is not None:
                            th()
                d2 = p_q.tile([128, S], bf, tag="q")
                for half in range(2):
                    nc.scalar.copy(d2[:, half * 512:(half + 1) * 512],
                                   qps[half][:])
                d1 = p_q.tile([128, S], bf, tag="q")
                rope_var(d1, d2)
                q1_t.append(d1)
                q2_t.append(d2)
                for th in zi:
                    th()

            q_proj(0)
            q_proj(1)
            q_proj(2)
            sc0 = score_mms(0)
            q_proj(3, zipped=sc0[:8])
            for th in sc0[8:]:
                th()

            # ---- V projection, zip scores h1 + h2 ----
            sc12 = score_mms(1) + score_mms(2)
            sci = 0
            v_t = []
            for sb in range(SB):
                pv = ps.tile([128, KPC * HD], f32, tag="pv", bufs=2,
                             name="vps")
                for t in range(KC):
                    nc.tensor.matmul(
                        pv[:],
                        lhsT=xts(t, sb * 128, (sb + 1) * 128),
                        rhs=wv_sb[:, t * 256:(t + 1) * 256],
                        start=(t == 0), stop=(t == KC - 1))
                tv = p_v.tile([128, 2 * (HD + 1)], bf, tag="v")
                nc.vector.tensor_copy(tv[:, 0:HD], pv[:, 0:HD])
                nc.vector.tensor_copy(tv[:, HD + 1:2 * HD + 1],
                                      pv[:, HD:2 * HD])
                nc.vector.memset(tv[:, HD:HD + 1], 1.0)
                nc.vector.memset(tv[:, 2 * HD + 1:2 * HD + 2], 1.0)
                v_t.append(tv)
                for _ in range(4):
                    if sci < len(sc12):
                        sc12[sci]()
                        sci += 1
            while sci < len(sc12):
                sc12[sci]()
                sci += 1

            # masks h0/h1 on vector (after v copies in queue order),
            # h2 on gpsimd (h3 must wait for the h3 score thunks below)
            for i in range(SB):
                for h in (0, 1):
                    emit_masks(i, h, nc.vector)
            for i in range(SB):
                emit_masks(i, 2, nc.gpsimd)

            an_t = {}

            def emit_attnv_mm(i, h):
                kv = h // 2
                pso = ps.tile([128, HD + 1], f32, tag="pa", name="pso")
                for j in range(i + 1):
                    nc.tensor.matmul(
                        pso[:], lhsT=P(i, h, j),
                        rhs=v_t[j][:, kv * (HD + 1):(kv + 1) * (HD + 1)],
                        start=(j == 0), stop=(j == i))
                rc = p_rc.tile([128, 1], f32, tag="rc")
                nc.vector.reciprocal(rc[:], pso[:, HD:HD + 1])
                an = p_an.tile([128, 128], bf, tag="an")
                nc.vector.tensor_scalar_mul(an[:], pso[:, 0:HD], rc[:])
                an_t[(i, h)] = an

            # ---- zip: scores h3 with attnv rows of h0/h1 ----
            sc3 = score_mms(3)
            av01 = [(i, h) for i in range(SB) for h in (0, 1)]
            avi = 0
            for th in sc3:
                th()
                if avi < len(av01):
                    emit_attnv_mm(*av01[avi])
                    avi += 1
            while avi < len(av01):
                emit_attnv_mm(*av01[avi])
                avi += 1

            for i in range(SB):
                emit_masks(i, 3, nc.gpsimd)

            # ---- row sweep: attnv h2/h3 + lagged transpose/out-proj ----
            def emit_transpose(i, h):
                pst = ps.tile([128, 128], bf, tag="pv", bufs=2, name="pst")
                nc.tensor.transpose(pst[:], an_t[(i, h)][:], id_t)
                if h < 2:
                    nc.scalar.copy(ao_t[h][:, i * 128:(i + 1) * 128], pst[:])
                else:
                    nc.vector.tensor_copy(
                        ao_t[h][:, i * 128:(i + 1) * 128], pst[:])

            def emit_outproj(i):
                st = p_st.tile([128, D], bf, tag="st")
                for cg in range(4):
                    po = ps.tile([128, 512], f32, tag="sc", bufs=2, name="po")
                    for hc in range(HPC):
                        nc.tensor.matmul(
                            po[:],
                            lhsT=ao_t[hc][:, i * 128:(i + 1) * 128],
                            rhs=wo_sb[:, hc * D + cg * 512:
                                      hc * D + (cg + 1) * 512],
                            start=(hc == 0), stop=(hc == HPC - 1))
                    if cg % 2 == 0:
                        nc.vector.tensor_copy(
                            st[:, cg * 512:(cg + 1) * 512], po[:])
                    else:
                        nc.scalar.copy(
                            st[:, cg * 512:(cg + 1) * 512], po[:])
                nc.sync.dma_start(out[i * 128:(i + 1) * 128, :], st[:])

            ao_t = [p_ao.tile([128, S], bf, tag="ao", name=f"ao{h}")
                    for h in range(HPC)]
            for i in range(SB):
                emit_attnv_mm(i, 2)
                emit_attnv_mm(i, 3)
                if i > 0:
                    for h in range(HPC):
                        emit_transpose(i - 1, h)
                    emit_outproj(i - 1)
            for h in range(HPC):
                emit_transpose(SB - 1, h)
            emit_outproj(SB - 1)

    nc.finalize()
    return nc


def _get_nc():
    if "nc" not in _NC_CACHE:
        _NC_CACHE["nc"] = _build_nc()
    return _NC_CACHE["nc"]


def _host_inputs(x, freqs_cos, freqs_sin, wq, wk, wv, wo):
    """Build the 8 per-core input maps (host-side shard + layout prep)."""
    x = np.asarray(x, np.float32)
    wq = np.asarray(wq, np.float32)
    wk = np.asarray(wk, np.float32)
    wv = np.asarray(wv, np.float32)
    wo = np.asarray(wo, np.float32)
    perm = np.concatenate([np.arange(0, HD, 2), np.arange(1, HD, 2)])

    cos_t = np.asarray(freqs_cos, np.float32).T        # (64, S)
    sin_t = np.asarray(freqs_sin, np.float32).T
    top = np.concatenate([cos_t, -sin_t], axis=1)      # (64, 2S)
    bot = np.concatenate([cos_t, sin_t], axis=1)
    tab = np.ascontiguousarray(np.concatenate([top, bot], axis=0)).astype(BF16)
    ki = np.arange(128)[:, None]
    qi = np.arange(128)[None, :]
    m0 = (ki <= qi).astype(BF16)                       # causal / far-select
    m2 = (qi < ki).astype(BF16)                        # in-band select (d=2)
    ident = np.eye(128, dtype=BF16)
    cstm = np.ascontiguousarray(np.concatenate([m0, m2, ident], axis=1))

    wq3 = wq.reshape(D, NH, HD)
    wk3 = wk.reshape(D, NKV, HD)
    wv3 = wv.reshape(D, NKV, HD)
    wo3 = wo.reshape(NH, HD, D)

    in_maps = []
    for c in range(8):
        b, g = divmod(c, 4)
        wqc = wq3[:, 4 * g:4 * g + 4][:, :, perm].reshape(D, HPC * HD)
        wkc = wk3[:, 2 * g:2 * g + 2][:, :, perm].reshape(D, KPC * HD)
        wvc = wv3[:, 2 * g:2 * g + 2].reshape(D, KPC * HD)
        woc = wo3[4 * g:4 * g + 4].reshape(HPC * HD, D)
        in_maps.append({
            "xt": np.ascontiguousarray(x[b].T).astype(BF16),
            "wq": np.ascontiguousarray(wqc).astype(BF16),
            "wk": np.ascontiguousarray(wkc).astype(BF16),
            "wv": np.ascontiguousarray(wvc).astype(BF16),
            "wo": np.ascontiguousarray(woc).astype(BF16),
            "tab": tab, "cst": cstm,
            "cw": np.ascontiguousarray(np.concatenate([
                np.stack([cos_t[:, W], sin_t[:, W]], axis=1),
                np.stack([cos_t[:, W], -sin_t[:, W]], axis=1),
            ], axis=0)).astype(np.float32),
        })
    return in_maps


def _run(nc, in_maps, **kw):
    from concourse.bass_utils import run_bass_kernel_spmd
    return run_bass_kernel_spmd(nc, in_maps, core_ids=list(range(8)), **kw)


def kernel(x, freqs_cos, freqs_sin, wq, wk, wv, wo):
    nc = _get_nc()
    in_maps = _host_inputs(x, freqs_cos, freqs_sin, wq, wk, wv, wo)
    res = _run(nc, in_maps)
    parts = [np.asarray(res.results[c]["out"], np.float32) for c in range(8)]
    out = np.stack([sum(parts[0:4]), sum(parts[4:8])])
    return out.astype(np.float32)


# revision 19
# speedup vs baseline: 1.0397x; 1.0001x over previous
"""Self-contained Trainium2 kernel for ReRoPE sparse attention.

Problem: x(2,1024,2048) -> attention with 16 Q heads / 8 KV heads (GQA),
RoPE within a 256-token causal band, ReRoPE (query rotated at fixed
position 256, keys unrotated) outside the band, -> out proj (2048x2048).

Sharding: 8 cores = 2 batches x 4 head groups. Each core computes 4 Q
heads / 2 KV heads of one batch plus its slice of all projections, and
produces a partial (1024,2048) output (wo row-parallel). Partials are
summed on the host (the per-batch all-reduce equivalent).

Score identity used: s2 = (R_W q)@k  ==  q @ (R_{-W} k), so the fixed
ReRoPE rotation is applied once to K instead of Q (q2 is just raw q).
Head dims are de-interleaved (evens|odds) via a host-side permutation of
wq/wk columns so RoPE pairs live on partitions (p, p+64).

v4 schedule: projections complete staggered (K, then Q heads h0..h3,
then V) so psum->sbuf copies and ropes overlap later projections;
key-block-grouped scores with wide query rhs stream zipped into the
Q(h3)/V matmul stream so the Scalar engine's exp pace is matched;
band/far select masks split across Vector (h0,h1) and Pool (h2,h3);
attention h0/h1 zipped behind the last scores; row sweep does attention
h2/h3 + lagged transposes + out-proj + one merged DMA per row.

PSUM tags: pa(4) K psums -> Q h2/h3 -> attn@V accumulators; sc(2)
Q h0 -> score psums -> out-proj; pv(2) Q h1 -> V -> transposes.

All device compute in bf16 (fp32 PSUM accumulation).
"""

import numpy as np
import ml_dtypes

B, S, D = 2, 1024, 2048
NH, NKV, HD = 16, 8, 128
W = 256
HPC, KPC = 4, 2            # q heads / kv heads per core
KC = D // 128              # 16 contraction chunks
SB = S // 128              # 8 sequence blocks
SCALE = 1.0 / float(np.sqrt(HD))
BF16 = ml_dtypes.bfloat16

# band strip: key block j covers queries j..min(j+2,7)
BW = [min(3, SB - j) for j in range(SB)]            # widths (blocks)
BOFF = np.cumsum([0] + BW).tolist()                 # block offsets
# far strip: key block j covers queries j+2..7
FW = [SB - 2 - j for j in range(SB - 2)]            # widths (blocks)
FOFF = np.cumsum([0] + FW).tolist()

_NC_CACHE = {}


def _build_nc():
    import concourse.bass as bass
    import concourse.tile as tile
    from concourse import bacc, mybir
    from contextlib import ExitStack

    bf = mybir.dt.bfloat16
    f32 = mybir.dt.float32
    AF = mybir.ActivationFunctionType
    MUL = mybir.AluOpType.mult
    SUB = mybir.AluOpType.subtract

    nc = bacc.Bacc()
    xt = nc.declare_dram_parameter("xt", [D, S], bf, isOutput=False)
    wq = nc.declare_dram_parameter("wq", [D, HPC * HD], bf, isOutput=False)
    wk = nc.declare_dram_parameter("wk", [D, KPC * HD], bf, isOutput=False)
    wv = nc.declare_dram_parameter("wv", [D, KPC * HD], bf, isOutput=False)
    wo = nc.declare_dram_parameter("wo", [HPC * HD, D], bf, isOutput=False)
    tab = nc.declare_dram_parameter("tab", [128, 2 * S], bf, isOutput=False)
    cst = nc.declare_dram_parameter("cst", [128, 3 * 128], bf, isOutput=False)
    cwd = nc.declare_dram_parameter("cw", [128, 2], f32, isOutput=False)
    out = nc.declare_dram_parameter("out", [S, D], bf, isOutput=True)

    with tile.TileContext(nc) as tc:
        with ExitStack() as ctx:
            p_x = ctx.enter_context(tc.tile_pool(name="p_x", bufs=1))
            p_w = ctx.enter_context(tc.tile_pool(name="p_w", bufs=1))
            p_tab = ctx.enter_context(tc.tile_pool(name="p_tab", bufs=1))
            p_q = ctx.enter_context(tc.tile_pool(name="p_q", bufs=2 * HPC))
            p_k = ctx.enter_context(tc.tile_pool(name="p_k", bufs=2 * KPC))
            p_v = ctx.enter_context(tc.tile_pool(name="p_v", bufs=SB))
            p_ao = ctx.enter_context(tc.tile_pool(name="p_ao", bufs=HPC))
            p_e = ctx.enter_context(tc.tile_pool(name="p_e", bufs=HPC))
            p_pt = ctx.enter_context(tc.tile_pool(name="p_pt", bufs=24))
            p_an = ctx.enter_context(tc.tile_pool(name="p_an", bufs=20))
            p_kr = ctx.enter_context(tc.tile_pool(name="p_kr", bufs=2))
            p_rt = ctx.enter_context(tc.tile_pool(name="p_rt", bufs=4))
            p_rc = ctx.enter_context(tc.tile_pool(name="p_rc", bufs=4))
            p_st = ctx.enter_context(tc.tile_pool(name="p_st", bufs=2))

            ps = ctx.enter_context(
                tc.tile_pool(name="ps", bufs=4, space="PSUM"))

            # ---- DMA schedule (x striped across both rings) ----
            xt_sb = p_x.tile([128, KC * S], bf, tag="xt")
            xt_d = xt.ap().rearrange("(t p) s -> p t s", p=128)
            xt_v = xt_sb[:].rearrange("p (t s) -> p t s", t=KC)
            wq_sb = p_w.tile([128, KC * HPC * HD], bf, tag="wq")
            wq_d = wq.ap().rearrange("(t p) c -> p t c", p=128)
            wq_v = wq_sb[:].rearrange("p (t c) -> p t c", t=KC)
            wk_sb = p_w.tile([128, KC * KPC * HD], bf, tag="wk")
            wk_d = wk.ap().rearrange("(t p) c -> p t c", p=128)
            wk_v = wk_sb[:].rearrange("p (t c) -> p t c", t=KC)
            wv_sb = p_w.tile([128, KC * KPC * HD], bf, tag="wv")
            wv_d = wv.ap().rearrange("(t p) c -> p t c", p=128)
            wv_v = wv_sb[:].rearrange("p (t c) -> p t c", t=KC)
            wo_sb = p_w.tile([128, HPC * D], bf, tag="wo")
            wo_d = wo.ap().rearrange("(t p) c -> p t c", p=128)
            wo_v = wo_sb[:].rearrange("p (t c) -> p t c", t=HPC)

            # ring A (sync): wk evens + even x chunks, then wq heads, wo
            # ring B (scalar): wk odds + odd x chunks, tables, wv
            def rA(dst, src):
                nc.sync.dma_start(dst, src)

            def rB(dst, src):
                nc.scalar.dma_start(dst, src)

            tab_sb = p_tab.tile([128, 2 * S], bf, tag="tab")
            cw_sb = p_tab.tile([128, 2], f32, tag="cw")
            cst_sb = p_tab.tile([128, 3 * 128], bf, tag="cst")
            rA(wk_v[:, 0:2, :], wk_d[:, 0:2, :])
            rB(xt_v[:, 1:2, :], xt_d[:, 1:2, :])
            rA(xt_v[:, 0:1, :], xt_d[:, 0:1, :])
            rB(wk_v[:, 2:4, :], wk_d[:, 2:4, :])
            rA(xt_v[:, 2:3, :], xt_d[:, 2:3, :])
            rB(xt_v[:, 3:4, :], xt_d[:, 3:4, :])
            rA(wk_v[:, 4:6, :], wk_d[:, 4:6, :])
            rB(xt_v[:, 5:6, :], xt_d[:, 5:6, :])
            rA(xt_v[:, 4:5, :], xt_d[:, 4:5, :])
            rB(wk_v[:, 6:8, :], wk_d[:, 6:8, :])
            rA(xt_v[:, 6:7, :], xt_d[:, 6:7, :])
            rB(xt_v[:, 7:8, :], xt_d[:, 7:8, :])
            rA(wk_v[:, 8:10, :], wk_d[:, 8:10, :])
            rB(xt_v[:, 9:10, :], xt_d[:, 9:10, :])
            rA(xt_v[:, 8:9, :], xt_d[:, 8:9, :])
            rB(wk_v[:, 10:12, :], wk_d[:, 10:12, :])
            rA(xt_v[:, 10:11, :], xt_d[:, 10:11, :])
            rB(xt_v[:, 11:12, :], xt_d[:, 11:12, :])
            rA(wk_v[:, 12:14, :], wk_d[:, 12:14, :])
            rB(xt_v[:, 13:14, :], xt_d[:, 13:14, :])
            rA(xt_v[:, 12:13, :], xt_d[:, 12:13, :])
            rB(wk_v[:, 14:16, :], wk_d[:, 14:16, :])
            rA(xt_v[:, 14:15, :], xt_d[:, 14:15, :])
            rB(xt_v[:, 15:16, :], xt_d[:, 15:16, :])
            for h in range(2):
                rA(wq_v[:, :, h * 128:(h + 1) * 128],
                   wq_d[:, :, h * 128:(h + 1) * 128])
            rB(tab_sb[:], tab[:, :])
            rB(cw_sb[:], cwd[:, :])
            for h in range(2, 4):
                rB(wq_v[:, :, h * 128:(h + 1) * 128],
                   wq_d[:, :, h * 128:(h + 1) * 128])
            rA(wo_v[:, 0:2, :], wo_d[:, 0:2, :])
            rA(wo_v[:, 2:4, :], wo_d[:, 2:4, :])
            rB(cst_sb[:], cst[:, :])
            rB(wv_v[:, 0:8, :], wv_d[:, 0:8, :])
            rB(wv_v[:, 8:16, :], wv_d[:, 8:16, :])

            def xts(t, lo, hi):
                return xt_sb[:, t * S + lo:t * S + hi]

            cosT = tab_sb[:, 0:S]
            sinT = tab_sb[:, S:2 * S]     # top half negated (host-side)
            m0_t = cst_sb[:, 0:128]      # (k <= q)
            m2_t = cst_sb[:, 128:256]    # (q < k)
            id_t = cst_sb[:, 256:384]

            def rope_var(dst, src):
                """Positional rope over full S; pairs on (p, p+64).
                Table halves carry signs: sinT[0:64]=-sin, [64:128]=+sin."""
                r, i = src[0:64, :], src[64:128, :]
                t1 = p_rt.tile([64, S], bf, tag="rt")
                nc.vector.tensor_mul(t1[:], r, cosT[0:64, :])
                t2 = p_rt.tile([64, S], bf, tag="rt")
                nc.vector.tensor_mul(t2[:], i, sinT[64:128, :])
                nc.vector.tensor_sub(dst[0:64, :], t1[:], t2[:])
                t3 = p_rt.tile([64, S], bf, tag="rt")
                nc.vector.tensor_mul(t3[:], r, sinT[0:64, :])    # = -r*sin
                t4 = p_rt.tile([64, S], bf, tag="rt")
                nc.vector.tensor_mul(t4[:], i, cosT[64:128, :])
                nc.vector.tensor_sub(dst[64:128, :], t4[:], t3[:])

            def rope_negw(dst, src):
                """R_{-W}: or = r*cw + i*sw, oi = i*cw - r*sw.
                cw col0 = [cw;cw], col1 = [+sw; -sw]. stt inputs must
                share a start partition (verifier), outputs may differ."""
                r, i = src[0:64, :], src[64:128, :]
                t2 = p_rt.tile([64, S], bf, tag="rt")
                nc.vector.tensor_scalar_mul(t2[:], i, cw_sb[64:128, 1:2])
                nc.vector.scalar_tensor_tensor(
                    dst[0:64, :], r, cw_sb[0:64, 0:1], t2[:], MUL, SUB)
                t4 = p_rt.tile([128, S], bf, tag="rt4", bufs=2)
                nc.vector.tensor_scalar_mul(t4[64:128, :], r,
                                            cw_sb[0:64, 1:2])
                nc.vector.scalar_tensor_tensor(
                    dst[64:128, :], i, cw_sb[64:128, 0:1], t4[64:128, :],
                    MUL, SUB)

            # ---- K projection (finishes early; ropes overlap Q) ----
            kps = [[ps.tile([128, 512], f32, tag="pa",
                            name=f"kps{kv}{half}") for half in range(2)]
                   for kv in range(KPC)]
            for t in range(KC):
                for kv in range(KPC):
                    for half in range(2):
                        nc.tensor.matmul(
                            kps[kv][half][:],
                            lhsT=wk_sb[:, t * 256 + kv * 128:
                                       t * 256 + (kv + 1) * 128],
                            rhs=xts(t, half * 512, (half + 1) * 512),
                            start=(t == 0), stop=(t == KC - 1))

            k1_t, k2_t = [], []
            for kv in range(KPC):
                kr = p_kr.tile([128, S], bf, tag="kr")
                for half in range(2):
                    nc.scalar.copy(kr[:, half * 512:(half + 1) * 512],
                                   kps[kv][half][:])
                d1 = p_k.tile([128, S], bf, tag="k")
                d2 = p_k.tile([128, S], bf, tag="k")
                rope_var(d1, kr)
                rope_negw(d2, kr)
                k1_t.append(d1)
                k2_t.append(d2)

            # ---- score machinery (key-block-grouped, wide query rhs) ----
            eb_t = {}
            ef_t = {}
            q1_t, q2_t = [], []

            for _h in range(HPC):
                eb_t[_h] = p_e.tile([128, BOFF[-1] * 128], bf, tag="eb",
                                    name=f"eb{_h}")
                ef_t[_h] = p_e.tile([128, FOFF[-1] * 128], bf, tag="ef",
                                    name=f"ef{_h}")

            def score_mms(h):
                """List of thunks, each emitting one band/far matmul+exp."""
                kv = h // 2
                ebs = eb_t[h]
                efs = ef_t[h]

                def band(j):
                    n = BW[j] * 128
                    pb = ps.tile([128, n], f32, tag="sc", bufs=2, name="scb")
                    nc.tensor.matmul(
                        pb[:], lhsT=k1_t[kv][:, j * 128:(j + 1) * 128],
                        rhs=q1_t[h][:, j * 128:j * 128 + n],
                        start=True, stop=True)
                    o = BOFF[j] * 128
                    nc.scalar.activation(ebs[:, o:o + n], pb[:],
                                         AF.Exp, scale=SCALE)

                def far(j, qlo, qn):
                    pb = ps.tile([128, qn * 128], f32, tag="sc", bufs=2,
                                 name="scf")
                    nc.tensor.matmul(
                        pb[:], lhsT=k2_t[kv][:, j * 128:(j + 1) * 128],
                        rhs=q2_t[h][:, qlo * 128:(qlo + qn) * 128],
                        start=True, stop=True)
                    o = (FOFF[j] + qlo - (j + 2)) * 128
                    nc.scalar.activation(efs[:, o:o + qn * 128], pb[:],
                                         AF.Exp, scale=SCALE)

                thunks = []
                for j in range(SB):
                    thunks.append(lambda j=j: band(j))
                    if j < SB - 2:
                        qlo = j + 2
                        while qlo < SB:
                            qn = min(4, SB - qlo)
                            thunks.append(
                                lambda j=j, qlo=qlo, qn=qn: far(j, qlo, qn))
                            qlo += qn
                return thunks

            def band_blk(h, j, i):
                o = (BOFF[j] + i - j) * 128
                return eb_t[h][:, o:o + 128]

            def far_blk(h, j, i):
                o = (FOFF[j] + i - (j + 2)) * 128
                return ef_t[h][:, o:o + 128]

            # masks: p0 = diag*m0 ; pd = band*m2 + far*m0
            pmask = {}

            def emit_masks(i, h, eng):
                p0 = p_pt.tile([128, 128], bf, tag="pt")
                eng.tensor_mul(p0[:], band_blk(h, i, i), m0_t)
                pd = None
                if i >= 2:
                    pa_ = p_pt.tile([128, 128], bf, tag="pt")
                    eng.tensor_mul(pa_[:], band_blk(h, i - 2, i), m2_t)
                    pd = p_pt.tile([128, 128], bf, tag="pt")
                    eng.tensor_mul(pd[:], far_blk(h, i - 2, i), m0_t)
                    eng.tensor_add(pd[:], pd[:], pa_[:])
                pmask[(i, h)] = (p0, pd)

            def P(i, h, j):
                p0, pd = pmask[(i, h)]
                if j == i:
                    return p0[:]
                if j == i - 2 and i >= 2:
                    return pd[:]
                if j >= i - 2:
                    return band_blk(h, j, i)
                return far_blk(h, j, i)

            # ---- Q projections, one head at a time ----
            QTAG = {0: ("sc", 2), 1: ("pv", 2), 2: ("pa", 4), 3: ("pa", 4)}

            def q_proj(h, zipped=()):
                tag, nb = QTAG[h]
                qps = [ps.tile([128, 512], f32, tag=tag, bufs=nb,
                               name=f"qps{h}{half}") for half in range(2)]
                zi = iter(zipped)
                for t in range(KC):
                    for half in range(2):
                        nc.tensor.matmul(
                            qps[half][:],
                            lhsT=wq_sb[:, t * 512 + h * 128:
                                       t * 512 + (h + 1) * 128],
                            rhs=xts(t, half * 512, (half + 1) * 512),
                            start=(t == 0), stop=(t == KC - 1))
                    if t % 2 == 1:
                        th = next(zi, None)
                        if th is not None:
                            th()
                d2 = p_q.tile([128, S], bf, tag="q")
                for half in range(2):
                    nc.scalar.copy(d2[:, half * 512:(half + 1) * 512],
                                   qps[half][:])
                d1 = p_q.tile([128, S], bf, tag="q")
                rope_var(d1, d2)
                q1_t.append(d1)
                q2_t.append(d2)
                for th in zi:
                    th()

            q_proj(0)
            q_proj(1)
            q_proj(2)
            sc0 = score_mms(0)
            q_proj(3, zipped=sc0[:8])
            for th in sc0[8:]:
                th()

            # ---- V projection, zip scores h1 + h2 ----
            sc12 = score_mms(1) + score_mms(2)
            sci = 0
            v_t = []
            for sb in range(SB):
                pv = ps.tile([128, KPC * HD], f32, tag="pv", bufs=2,
                             name="vps")
                for t in range(KC):
                    nc.tensor.matmul(
                        pv[:],
                        lhsT=xts(t, sb * 128, (sb + 1) * 128),
                        rhs=wv_sb[:, t * 256:(t + 1) * 256],
                        start=(t == 0), stop=(t == KC - 1))
                tv = p_v.tile([128, 2 * (HD + 1)], bf, tag="v")
                nc.vector.tensor_copy(tv[:, 0:HD], pv[:, 0:HD])
                nc.vector.tensor_copy(tv[:, HD + 1:2 * HD + 1],
                                      pv[:, HD:2 * HD])
                nc.vector.memset(tv[:, HD:HD + 1], 1.0)
                nc.vector.memset(tv[:, 2 * HD + 1:2 * HD + 2], 1.0)
                v_t.append(tv)
                for _ in range(4):
                    if sci < len(sc12):
                        sc12[sci]()
                        sci += 1
            while sci < len(sc12):
                sc12[sci]()
                sci += 1

            # masks h0/h1 on vector (after v copies in queue order),
            # h2 on gpsimd (h3 must wait for the h3 score thunks below)
            for i in range(SB):
                for h in (0, 1):
                    emit_masks(i, h, nc.vector)
            for i in range(SB):
                emit_masks(i, 2, nc.gpsimd)

            an_t = {}

            def emit_attnv_mm(i, h):
                kv = h // 2
                pso = ps.tile([128, HD + 1], f32, tag="pa", name="pso")
                for j in range(i + 1):
                    nc.tensor.matmul(
                        pso[:], lhsT=P(i, h, j),
                        rhs=v_t[j][:, kv * (HD + 1):(kv + 1) * (HD + 1)],
                        start=(j == 0), stop=(j == i))
                rc = p_rc.tile([128, 1], f32, tag="rc")
                nc.vector.reciprocal(rc[:], pso[:, HD:HD + 1])
                an = p_an.tile([128, 128], bf, tag="an")
                nc.vector.tensor_scalar_mul(an[:], pso[:, 0:HD], rc[:])
                an_t[(i, h)] = an

            # ---- zip: scores h3 (tight, exp-paced) + early attnv rows ----
            sc3 = score_mms(3)
            av01 = [(i, h) for i in range(SB) for h in (0, 1)]
            avi = 0
            for n, th in enumerate(sc3):
                th()
                if n % 2 == 1 and avi < 8:
                    emit_attnv_mm(*av01[avi])
                    avi += 1
            while avi < len(av01):
                emit_attnv_mm(*av01[avi])
                avi += 1

            for i in range(SB):
                emit_masks(i, 3, nc.gpsimd)

            # ---- row sweep: attnv h2/h3 + lagged transpose/out-proj ----
            def emit_transpose(i, h):
                pst = ps.tile([128, 128], bf, tag="pv", bufs=2, name="pst")
                nc.tensor.transpose(pst[:], an_t[(i, h)][:], id_t)
                if h < 2:
                    nc.scalar.copy(ao_t[h][:, i * 128:(i + 1) * 128], pst[:])
                else:
                    nc.vector.tensor_copy(
                        ao_t[h][:, i * 128:(i + 1) * 128], pst[:])

            def emit_outproj(i):
                st = p_st.tile([128, D], bf, tag="st")
                for cg in range(4):
                    po = ps.tile([128, 512], f32, tag="sc", bufs=2, name="po")
                    for hc in range(HPC):
                        nc.tensor.matmul(
                            po[:],
                            lhsT=ao_t[hc][:, i * 128:(i + 1) * 128],
                            rhs=wo_sb[:, hc * D + cg * 512:
                                      hc * D + (cg + 1) * 512],
                            start=(hc == 0), stop=(hc == HPC - 1))
                    if cg % 2 == 0:
                        nc.vector.tensor_copy(
                            st[:, cg * 512:(cg + 1) * 512], po[:])
                    else:
                        nc.scalar.copy(
                            st[:, cg * 512:(cg + 1) * 512], po[:])
                nc.sync.dma_start(out[i * 128:(i + 1) * 128, :], st[:])

            ao_t = [p_ao.tile([128, S], bf, tag="ao", name=f"ao{h}")
                    for h in range(HPC)]
            for i in range(SB):
                emit_attnv_mm(i, 2)
                emit_attnv_mm(i, 3)
                if i > 1:
                    for h in range(HPC):
                        emit_transpose(i - 2, h)
                    emit_outproj(i - 2)
            for i in (SB - 2, SB - 1):
                for h in range(HPC):
                    emit_transpose(i, h)
                emit_outproj(i)

    nc.finalize()
    return nc


def _get_nc():
    if "nc" not in _NC_CACHE:
        _NC_CACHE["nc"] = _build_nc()
    return _NC_CACHE["nc"]


def _host_inputs(x, freqs_cos, freqs_sin, wq, wk, wv, wo):
    """Build the 8 per-core input maps (host-side shard + layout prep)."""
    x = np.asarray(x, np.float32)
    wq = np.asarray(wq, np.float32)
    wk = np.asarray(wk, np.float32)
    wv = np.asarray(wv, np.float32)
    wo = np.asarray(wo, np.float32)
    perm = np.concatenate([np.arange(0, HD, 2), np.arange(1, HD, 2)])

    cos_t = np.asarray(freqs_cos, np.float32).T        # (64, S)
    sin_t = np.asarray(freqs_sin, np.float32).T
    top = np.concatenate([cos_t, -sin_t], axis=1)      # (64, 2S)
    bot = np.concatenate([cos_t, sin_t], axis=1)
    tab = np.ascontiguousarray(np.concatenate([top, bot], axis=0)).astype(BF16)
    ki = np.arange(128)[:, None]
    qi = np.arange(128)[None, :]
    m0 = (ki <= qi).astype(BF16)                       # causal / far-select
    m2 = (qi < ki).astype(BF16)                        # in-band select (d=2)
    ident = np.eye(128, dtype=BF16)
    cstm = np.ascontiguousarray(np.concatenate([m0, m2, ident], axis=1))

    wq3 = wq.reshape(D, NH, HD)
    wk3 = wk.reshape(D, NKV, HD)
    wv3 = wv.reshape(D, NKV, HD)
    wo3 = wo.reshape(NH, HD, D)

    in_maps = []
    for c in range(8):
        b, g = divmod(c, 4)
        wqc = wq3[:, 4 * g:4 * g + 4][:, :, perm].reshape(D, HPC * HD)
        wkc = wk3[:, 2 * g:2 * g + 2][:, :, perm].reshape(D, KPC * HD)
        wvc = wv3[:, 2 * g:2 * g + 2].reshape(D, KPC * HD)
        woc = wo3[4 * g:4 * g + 4].reshape(HPC * HD, D)
        in_maps.append({
            "xt": np.ascontiguousarray(x[b].T).astype(BF16),
            "wq": np.ascontiguousarray(wqc).astype(BF16),
            "wk": np.ascontiguousarray(wkc).astype(BF16),
            "wv": np.ascontiguousarray(wvc).astype(BF16),
            "wo": np.ascontiguousarray(woc).astype(BF16),
            "tab": tab, "cst": cstm,
            "cw": np.ascontiguousarray(np.concatenate([
                np.stack([cos_t[:, W], sin_t[:, W]], axis=1),
                np.stack([cos_t[:, W], -sin_t[:, W]], axis=1),
            ], axis=0)).astype(np.float32),
        })
    return in_maps


def _run(nc, in_maps, **kw):
    from concourse.bass_utils import run_bass_kernel_spmd
    return run_bass_kernel_spmd(nc, in_maps, core_ids=list(range(8)), **kw)


def kernel(x, freqs_cos, freqs_sin, wq, wk, wv, wo):
    nc = _get_nc()
    in_maps = _host_inputs(x, freqs_cos, freqs_sin, wq, wk, wv, wo)
    res = _run(nc, in_maps)
    parts = [np.asarray(res.results[c]["out"], np.float32) for c in range(8)]
    out = np.stack([sum(parts[0:4]), sum(parts[4:8])])
    return out.astype(np.float32)
